# revision 1
# baseline (speedup 1.0000x reference)
"""Multi-head causal self-attention (V=Q variant) on 8 Trainium2 cores.

Sharding: batch (2) x head-group (4 groups of 4 heads). Each core computes
full-sequence attention for its 4 heads of one batch element, plus its slice
of the output projection; the host sums the 4 partial projections per batch
and adds b0.

Per core: xT [1024, 2048], Wq_s/Wk_s [1024, 256], W0_s [256, 1024].

Scores are computed transposed (S^T[kv, q]) so the softmax denominator falls
out of the AV matmul via a ones-column appended to V (V aliases Q in this
module -- the reference replicates that bug). The 1/sqrt(DK) scale is folded
into Wk/bk on the host. All matmul inputs live in float32r SBUF tiles (TF32-
like, 1 cycle/row on the PE vs 4 for fp32); PSUM accumulation stays fp32.

ACT (exp) is the long-pole engine, so program order interleaves projection
sub-sweeps with attention j-blocks to start the exp stream as early as
possible. DVE does PSUM evictions + normalize, Pool does masks/broadcasts.
Causal diagonal tiles are narrowed to their valid q-range (left-trimmed).
"""

import ml_dtypes
import numpy as np

import concourse.bacc as bacc
import concourse.mybir as mybir
from concourse.tile import TileContext, add_dep_helper

P = 128
S = 2048  # sequence length
D = 1024  # model dim
HD = 256  # head-group width (4 heads x 64)
DK = 64
NQ = 4  # q chunks of 512
NKV = 16  # kv chunks of 128
NKD = 8  # D chunks of 128
F32 = mybir.dt.float32
F32R = mybir.dt.float32r
BF16 = mybir.dt.bfloat16
EXP = mybir.ActivationFunctionType.Exp

_CACHED_NC = None


def build_nc():
    nc = bacc.Bacc("TRN2", target_bir_lowering=False, debug=False, num_devices=8)
    xT = nc.declare_dram_parameter("xT", [D, S], BF16, isOutput=False)
    Wq = nc.declare_dram_parameter("Wq", [D, HD], BF16, isOutput=False)
    Wk = nc.declare_dram_parameter("Wk", [D, HD], BF16, isOutput=False)
    bqt = nc.declare_dram_parameter("bqt", [P, 2], F32, isOutput=False)
    bkt = nc.declare_dram_parameter("bkt", [P, 2], F32, isOutput=False)
    W0 = nc.declare_dram_parameter("W0", [HD, D], F32, isOutput=False)
    out = nc.declare_dram_parameter("out", [S, D], BF16, isOutput=True)

    def rd(ap):
        # reinterpret a DRAM f32 region as f32r for raw DMA into f32r tiles
        return ap.bitcast(F32R)

    with TileContext(nc) as tc:
        with (
            tc.tile_pool(name="const", bufs=1) as const,
            tc.tile_pool(name="xt", bufs=16) as xtp,
            tc.tile_pool(name="wqk", bufs=1) as wp,
            tc.tile_pool(name="vp", bufs=32) as vpool,
            tc.tile_pool(name="pt", bufs=6) as ptp,
            tc.tile_pool(name="dp", bufs=4) as dpool,
            tc.tile_pool(name="ost", bufs=3) as ostp,
            tc.tile_pool(name="mm", bufs=2, space="PSUM") as mmp,
            tc.tile_pool(name="sps", bufs=2, space="PSUM") as spsum,
            tc.tile_pool(name="aps", bufs=2, space="PSUM") as apsum,
        ):
            identity = const.tile([P, P], F32)
            nc.gpsimd.memset(identity[:], 0.0)
            nc.gpsimd.affine_select(
                out=identity[:],
                in_=identity[:],
                compare_op=mybir.AluOpType.not_equal,
                fill=1.0,
                base=0,
                pattern=[[-1, P]],
                channel_multiplier=1,
            )
            ones_col = const.tile([P, 1], F32)
            nc.gpsimd.memset(ones_col[:], 1.0)
            # triangular mask [128,128]: keep (1.0) where q >= kv, i.e. f >= p
            tri = const.tile([P, P], F32, name="tri")
            nc.gpsimd.memset(tri[:], 1.0)
            nc.gpsimd.affine_select(
                out=tri[:],
                in_=tri[:],
                compare_op=mybir.AluOpType.is_ge,
                fill=0.0,
                base=0,
                pattern=[[1, P]],
                channel_multiplier=-1,
            )
            # [128,256] mask: zeros block then triangle (for left-padded tiles)
            ztri = const.tile([P, 256], F32, name="ztri")
            nc.gpsimd.memset(ztri[:], 1.0)
            nc.gpsimd.affine_select(
                out=ztri[:],
                in_=ztri[:],
                compare_op=mybir.AluOpType.is_ge,
                fill=0.0,
                base=-128,
                pattern=[[1, 256]],
                channel_multiplier=-1,
            )
            # ACT exp-table warmup while DMAs run
            warm = const.tile([P, 8], F32, name="warm")
            nc.gpsimd.memset(warm[:], 0.0)
            nc.scalar.activation(out=warm[:], in_=warm[:], func=EXP)

            bq_sb = const.tile([P, 2], F32)
            bk_sb = const.tile([P, 2], F32)
            w0_sb = [const.tile([P, D], F32R, name=f"w0_{kc}") for kc in range(2)]
            # QT/KT as [mi][ni] tiles of [128, 512] for fine-grained deps
            QT = [
                [const.tile([P, 512], F32R, name=f"qt{mi}_{ni}") for ni in range(NQ)]
                for mi in range(2)
            ]
            KT = [
                [const.tile([P, 512], F32R, name=f"kt{mi}_{ni}") for ni in range(NQ)]
                for mi in range(2)
            ]
            # normalized attention (transposed), per q-chunk and head-pair
            attn = [
                [const.tile([P, 512], F32R, name=f"attn{j}_{p}") for p in range(2)]
                for j in range(4)
            ]

            # weights in one strided DMA each (per-DMA issue cadence is ~650ns,
            # so many small DMAs would serialize the stream)
            wq_big = wp.tile([P, NKD, HD], BF16, name="wqb")
            nc.sync.dma_start(out=wq_big[:], in_=Wq.rearrange("(k p) c -> p k c", p=P))
            wk_big = wp.tile([P, NKD, HD], BF16, name="wkb")
            nc.sync.dma_start(out=wk_big[:], in_=Wk.rearrange("(k p) c -> p k c", p=P))
            wq_t = [wq_big[:, k, :] for k in range(NKD)]
            wk_t = [wk_big[:, k, :] for k in range(NKD)]
            nc.sync.dma_start(out=bq_sb[:], in_=bqt[:, :])
            nc.sync.dma_start(out=bk_sb[:], in_=bkt[:, :])
            # xT as [k][half] tiles of [128, 1024]; 4 serialized chains so
            # all half-0 tiles (cols 0:1024) land before any half-1.
            xh = [
                [xtp.tile([P, 1024], BF16, name="xtile") for _ in range(2)]
                for _ in range(NKD)
            ]
            # x half-0 chunks first, then half-1, then W0: the DMA path
            # drains in issue order, which staggers arrivals naturally
            for h in range(2):
                for k in range(NKD):
                    nc.sync.dma_start(
                        out=xh[k][h][:],
                        in_=xT[k * P : (k + 1) * P, h * 1024 : (h + 1) * 1024],
                    )
            for kc in range(2):
                nc.sync.dma_start(
                    out=w0_sb[kc][:], in_=rd(W0[kc * P : (kc + 1) * P, :])
                )

            def sweep_items(ni, mi):
                """Projection sub-sweep as a list of emit-thunks (per-k)."""
                half, col = divmod(ni, 2)
                pss = [mmp.tile([P, 512], F32, name="ps") for _ in range(2)]

                def mk(k):
                    def go():
                        for ps, wt in zip(pss, (wq_t, wk_t)):
                            nc.tensor.matmul(
                                ps[:],
                                lhsT=wt[k][:, mi * P : (mi + 1) * P],
                                rhs=xh[k][half][:, col * 512 : (col + 1) * 512],
                                start=(k == 0),
                                stop=(k == NKD - 1),
                            )
                    return go

                def evict():
                    for ps, bias, dstT in zip(pss, (bq_sb, bk_sb), (QT, KT)):
                        nc.vector.tensor_scalar_add(
                            dstT[mi][ni][:, :], ps[:], bias[:, mi : mi + 1]
                        )

                return [mk(k) for k in range(NKD)] + [evict]

            vp = {}

            def emit_transposes(pair, i_lo, i_hi):
                # V' tiles [128, 132]: A data 0:64, A one 64, B data 66:130, B one 130
                for i in range(i_lo, i_hi):
                    tp = spsum.tile([P, P], F32, name="spsA")
                    nc.tensor.transpose(
                        tp[:, 0:P],
                        QT[pair][i // 4][:, (i % 4) * P : (i % 4 + 1) * P].bitcast(F32),
                        identity[:],
                    )
                    vt = vpool.tile([P, 132], BF16, name="vt")
                    nc.vector.tensor_copy(vt[:, 0:64], tp[:, 0:64])
                    nc.vector.tensor_copy(vt[:, 66:130], tp[:, 64:128])
                    nc.gpsimd.tensor_copy(vt[:, 64:65], ones_col[:])
                    nc.gpsimd.tensor_copy(vt[:, 130:131], ones_col[:])
                    vp[(pair, i)] = vt

            bg = []  # drip queue of (cost, thunk): sweeps, then phase-C blocks

            def drip(budget):
                while bg and budget > 0:
                    cost, thunk = bg.pop(0)
                    thunk()
                    budget -= cost

            def emit_cblock_m(j, c):
                m = j * 4 + c
                ot = ostp.tile([P, D], BF16, name="ot")
                for n in range(2):
                    ps = mmp.tile([P, 512], F32, name="ps")
                    for kc in range(2):
                        nc.tensor.matmul(
                            ps[:],
                            lhsT=attn[j][kc][:, c * P : (c + 1) * P],
                            rhs=w0_sb[kc][:, n * 512 : (n + 1) * 512],
                            start=(kc == 0),
                            stop=(kc == 1),
                        )
                    nc.vector.tensor_copy(ot[:, n * 512 : (n + 1) * 512], ps[:])
                nc.sync.dma_start(out=out[m * P : (m + 1) * P, :], in_=ot[:])

            def emit_pair(pair):
                steps = [(j, i) for j in range(NQ) for i in range(4 * j + 4)]
                ats = {}
                pend = None

                def emit_S(j, i):
                    off = max(0, i * P - j * 512)  # 0,128,256,384
                    w = 512 - off
                    # f32r matmuls need free>=256: pad the S matmul leftward
                    # for the 128-wide case, but exp/AV (bf16) use true width
                    swoff, swm = (256, 256) if w == P else (off, w)
                    kc = slice((i % 4) * P, (i % 4 + 1) * P)
                    sA = spsum.tile([P, 512], F32, name="spsA")
                    sB = spsum.tile([P, 512], F32, name="spsB")
                    nc.tensor.matmul(
                        sA[:, 0:swm],
                        lhsT=KT[pair][i // 4][0:64, kc],
                        rhs=QT[pair][j][0:64, swoff : swoff + swm],
                    )
                    nc.tensor.matmul(
                        sB[:, 0:swm],
                        lhsT=KT[pair][i // 4][64:128, kc],
                        rhs=QT[pair][j][64:128, swoff : swoff + swm],
                    )
                    qsl = slice(off, 512)
                    sskip = off - swoff  # valid region offset within s psum
                    pA = ptp.tile([P, 512], BF16, name="ptA")
                    pB = ptp.tile([P, 512], BF16, name="ptB")
                    nc.scalar.activation(
                        out=pA[:, 0:w], in_=sA[:, sskip : sskip + w], func=EXP
                    )
                    nc.scalar.activation(
                        out=pB[:, 0:w], in_=sB[:, sskip : sskip + w], func=EXP
                    )
                    if i >= 4 * j:  # diagonal tile: mask the leading block
                        nc.vector.tensor_mul(pA[:, 0:P], pA[:, 0:P], tri[:])
                        nc.vector.tensor_mul(pB[:, 0:P], pB[:, 0:P], tri[:])
                    return (j, i, pA, pB, qsl, w)

                def emit_AV(j, i, pA, pB, qsl, wm):
                    if i == 0:
                        ats[j] = (
                            apsum.tile([P, 512], F32, name="aps"),
                            apsum.tile([P, 512], F32, name="aps"),
                        )
                    atA, atB = ats[j]
                    imax = 4 * j + 3
                    vt = vp[(pair, i)]
                    nc.tensor.matmul(
                        atA[0:65, qsl],
                        lhsT=vt[:, 0:65],
                        rhs=pA[:, 0:wm],
                        start=(i == 0),
                        stop=(i == imax),
                    )
                    nc.tensor.matmul(
                        atB[0:65, qsl],
                        lhsT=vt[:, 66:131],
                        rhs=pB[:, 0:wm],
                        start=(i == 0),
                        stop=(i == imax),
                    )
                    if i == imax:  # normalize: attn = att_un / d, d = row 64
                        for at, rows in ((atA, slice(0, 64)), (atB, slice(64, 128))):
                            rec = dpool.tile([1, 512], F32, name="rec")
                            nc.vector.reciprocal(rec[:], at[64:65, :])
                            rbc = dpool.tile([64, 512], F32, name="rbc")
                            nc.gpsimd.partition_broadcast(rbc[0:64, :], rec[0:1, :])
                            nc.vector.tensor_mul(
                                attn[j][pair][rows, :], at[0:64, :], rbc[0:64, :]
                            )
                        if pair == 1:  # output projection becomes available
                            for c in range(4):
                                bg.append((1, lambda j=j, c=c: emit_cblock_m(j, c)))

                for j, i in steps:
                    cur = emit_S(j, i)
                    drip(5)
                    if pend is not None:
                        emit_AV(*pend)
                    pend = cur
                emit_AV(*pend)

            def t_item(pair, i):
                return (1, lambda: emit_transposes(pair, i, i + 1))

            # upfront: pair-0 ni=0 projection (DMA-paced) + first V transposes
            for it in sweep_items(0, 0):
                it()
            emit_transposes(0, 0, 4)
            # bg order follows need-by and DMA-arrival order; cost 2 paces the
            # first half-1-gated sweep to the chunk arrival rate
            for ni, mi in ((1, 0),):
                bg.extend((1, it) for it in sweep_items(ni, mi))
            bg.extend(t_item(0, i) for i in range(4, 8))
            bg.extend((1, it) for it in sweep_items(2, 0))
            bg.extend(t_item(0, i) for i in range(8, 12))
            bg.extend((1, it) for it in sweep_items(3, 0))
            bg.extend(t_item(0, i) for i in range(12, 16))
            bg.extend((1, it) for it in sweep_items(0, 1))
            bg.extend((1, it) for it in sweep_items(1, 1))
            bg.extend((1, it) for it in sweep_items(2, 1))
            bg.extend((1, it) for it in sweep_items(3, 1))
            bg.extend(t_item(1, i) for i in range(0, 16))
            emit_pair(0)
            emit_pair(1)
            while bg:
                drip(5)

    nc.compile()
    return nc


def make_in_maps(pos_encode_toks, Wq, bq, Wk, bk, W0, b0):
    x = np.asarray(pos_encode_toks, dtype=np.float32)
    Wq = np.asarray(Wq, dtype=np.float32)
    bq = np.asarray(bq, dtype=np.float32)
    Wk = np.asarray(Wk, dtype=np.float32)
    bk = np.asarray(bk, dtype=np.float32)
    W0 = np.asarray(W0, dtype=np.float32)
    in_maps = []
    for core in range(8):
        b, g = divmod(core, 4)
        hs = slice(g * HD, (g + 1) * HD)
        scale = np.float32(1.0 / np.sqrt(DK))
        in_maps.append(
            {
                "xT": np.ascontiguousarray(x[b].T).astype(ml_dtypes.bfloat16),
                "Wq": np.ascontiguousarray(Wq[:, hs]).astype(ml_dtypes.bfloat16),
                "Wk": np.ascontiguousarray(Wk[:, hs] * scale).astype(ml_dtypes.bfloat16),
                "bqt": np.ascontiguousarray(bq[hs].reshape(2, P).T),
                "bkt": np.ascontiguousarray((bk[hs] * scale).reshape(2, P).T),
                "W0": np.ascontiguousarray(W0[hs, :]),
            }
        )
    return in_maps


def assemble(results, b0):
    out = np.zeros((2, S, D), dtype=np.float32)
    for core in range(8):
        b = core // 4
        out[b] += results[core]["out"].astype(np.float32)
    out += np.asarray(b0, dtype=np.float32)
    return out


def kernel(pos_encode_toks, Wq, bq, Wk, bk, W0, b0):
    from concourse.bass_utils import run_bass_kernel_spmd

    global _CACHED_NC
    if _CACHED_NC is None:
        _CACHED_NC = build_nc()
    in_maps = make_in_maps(pos_encode_toks, Wq, bq, Wk, bk, W0, b0)
    res = run_bass_kernel_spmd(_CACHED_NC, in_maps, core_ids=list(range(8)))
    return assemble(res.results, b0)



# revision 26
# speedup vs baseline: 1.1271x; 1.1271x over previous
"""Multi-head causal self-attention (V=Q variant) on 8 Trainium2 cores.

Sharding: batch (2) x head-group (4 groups of 4 heads). Each core computes
full-sequence attention for its 4 heads of one batch element, plus its slice
of the output projection; the host sums the 4 partial projections per batch
and adds b0.

Per core: xT [1024, 2048], Wq_s/Wk_s [1024, 256], W0_s [256, 1024].

Scores are computed transposed (S^T[kv, q]) so the softmax denominator falls
out of the AV matmul via a ones-column appended to V (V aliases Q in this
module -- the reference replicates that bug). The 1/sqrt(DK) scale is folded
into Wk/bk on the host. All matmul operands are bf16 (1 cycle/row on the PE);
PSUM accumulation stays fp32.

The two heads of a pair share one [128,1024] score PSUM tile so a single ACT
exp instruction covers both (halves ACT instruction count). V' tiles are
produced by DMA-engine transposes (XBAR) straight from the bf16 QT tiles,
keeping the PE out of the transpose path and DVE out of the evictions. ACT
(exp) is the attention-phase rate limiter, so program order interleaves
projection sub-sweeps and output-projection blocks into the PE's exp-wait
gaps via a drip queue. Causal diagonal tiles are narrowed to their valid
q-range (left-trimmed).
"""

import ml_dtypes
import numpy as np

import concourse.bacc as bacc
import concourse.mybir as mybir
from concourse.tile import TileContext

P = 128
S = 2048  # sequence length
D = 1024  # model dim
HD = 256  # head-group width (4 heads x 64)
DK = 64
NQ = 4  # q chunks of 512
NKV = 16  # kv chunks of 128
NKD = 8  # D chunks of 128
F32 = mybir.dt.float32
BF16 = mybir.dt.bfloat16
EXP = mybir.ActivationFunctionType.Exp

_CACHED_NC = None


def build_nc():
    nc = bacc.Bacc("TRN2", target_bir_lowering=False, debug=False, num_devices=8)
    xT = nc.declare_dram_parameter("xT", [D, S], BF16, isOutput=False)
    Wq = nc.declare_dram_parameter("Wq", [D, HD], BF16, isOutput=False)
    Wk = nc.declare_dram_parameter("Wk", [D, HD], BF16, isOutput=False)
    bqt = nc.declare_dram_parameter("bqt", [P, 2], F32, isOutput=False)
    bkt = nc.declare_dram_parameter("bkt", [P, 2], F32, isOutput=False)
    W0 = nc.declare_dram_parameter("W0", [HD, D], BF16, isOutput=False)
    out = nc.declare_dram_parameter("out", [S, D], BF16, isOutput=True)

    with TileContext(nc) as tc:
        with (
            tc.tile_pool(name="const", bufs=1) as const,
            tc.tile_pool(name="xt", bufs=16) as xtp,
            tc.tile_pool(name="wqk", bufs=1) as wp,
            tc.tile_pool(name="vp", bufs=32) as vpool,
            tc.tile_pool(name="pt", bufs=6) as ptp,
            tc.tile_pool(name="dp", bufs=4) as dpool,
            tc.tile_pool(name="ost", bufs=3) as ostp,
            tc.tile_pool(name="mm", bufs=2, space="PSUM") as mmp,
            tc.tile_pool(name="sps", bufs=2, space="PSUM") as spsum,
            tc.tile_pool(name="aps", bufs=2, space="PSUM") as apsum,
        ):
            ones_col = const.tile([P, 1], BF16)
            nc.gpsimd.memset(ones_col[:], 1.0)
            identity = const.tile([P, P], BF16)
            nc.gpsimd.memset(identity[:], 0.0)
            nc.gpsimd.affine_select(
                out=identity[:],
                in_=identity[:],
                compare_op=mybir.AluOpType.not_equal,
                fill=1.0,
                base=0,
                pattern=[[-1, P]],
                channel_multiplier=1,
            )
            # triangular mask [128,128] bf16: keep (1.0) where q >= kv
            tri = const.tile([P, P], BF16, name="tri")
            nc.gpsimd.memset(tri[:], 1.0)
            nc.gpsimd.affine_select(
                out=tri[:],
                in_=tri[:],
                compare_op=mybir.AluOpType.is_ge,
                fill=0.0,
                base=0,
                pattern=[[1, P]],
                channel_multiplier=-1,
            )
            # ACT exp-table warmup while DMAs run
            warm = const.tile([P, 8], F32, name="warm")
            nc.gpsimd.memset(warm[:], 0.0)
            nc.scalar.activation(out=warm[:], in_=warm[:], func=EXP)

            bq_sb = const.tile([P, 2], F32)
            bk_sb = const.tile([P, 2], F32)
            w0_sb = [const.tile([P, D], BF16, name=f"w0_{kc}") for kc in range(2)]
            # QT/KT as [mi][ni] tiles of [128, 512] for fine-grained deps
            QT = [
                [const.tile([P, 512], BF16, name=f"qt{mi}_{ni}") for ni in range(NQ)]
                for mi in range(2)
            ]
            KT = [
                [const.tile([P, 512], BF16, name=f"kt{mi}_{ni}") for ni in range(NQ)]
                for mi in range(2)
            ]
            # normalized attention (transposed), per q-chunk and head-pair
            attn = [
                [const.tile([P, 512], BF16, name=f"attn{j}_{p}") for p in range(2)]
                for j in range(4)
            ]

            # first k-slice of each weight as its own tile/DMA so the first
            # projection sub-sweep starts as soon as ~130KB has landed
            wq_t0 = wp.tile([P, HD], BF16, name="wq0")
            wk_t0 = wp.tile([P, HD], BF16, name="wk0")
            wq_r = wp.tile([P, NKD - 1, HD], BF16, name="wqr")
            wk_r = wp.tile([P, NKD - 1, HD], BF16, name="wkr")
            wq_view = Wq.rearrange("(k p) c -> p k c", p=P)
            wk_view = Wk.rearrange("(k p) c -> p k c", p=P)
            nc.sync.dma_start(out=wq_t0[:], in_=wq_view[:, 0, :])
            xh = [
                [xtp.tile([P, 1024], BF16, name="xtile") for _ in range(2)]
                for _ in range(NKD)
            ]
            nc.sync.dma_start(out=xh[0][0][:], in_=xT[0:P, 0:1024])
            nc.sync.dma_start(out=wk_t0[:], in_=wk_view[:, 0, :])
            nc.sync.dma_start(out=bq_sb[:], in_=bqt[:, :])
            nc.sync.dma_start(out=bk_sb[:], in_=bkt[:, :])
            nc.sync.dma_start(out=wq_r[:], in_=wq_view[:, 1:NKD, :])
            nc.sync.dma_start(out=wk_r[:], in_=wk_view[:, 1:NKD, :])
            wq_t = [wq_t0[:]] + [wq_r[:, k - 1, :] for k in range(1, NKD)]
            wk_t = [wk_t0[:]] + [wk_r[:, k - 1, :] for k in range(1, NKD)]
            # x half-0 chunks first, then half-1, then W0: the DMA path
            # drains in issue order, which staggers arrivals naturally
            for h in range(2):
                for k in range(NKD):
                    if h == 0 and k == 0:
                        continue
                    nc.sync.dma_start(
                        out=xh[k][h][:],
                        in_=xT[k * P : (k + 1) * P, h * 1024 : (h + 1) * 1024],
                    )
            for kc in range(2):
                nc.sync.dma_start(out=w0_sb[kc][:], in_=W0[kc * P : (kc + 1) * P, :])

            def sweep_items(ni, mi):
                """Projection sub-sweep as a list of emit-thunks (per-k)."""
                half, col = divmod(ni, 2)
                pss = [mmp.tile([P, 512], F32, name="ps") for _ in range(2)]

                def mk(k):
                    def go():
                        for ps, wt in zip(pss, (wq_t, wk_t)):
                            nc.tensor.matmul(
                                ps[:],
                                lhsT=wt[k][:, mi * P : (mi + 1) * P],
                                rhs=xh[k][half][:, col * 512 : (col + 1) * 512],
                                start=(k == 0),
                                stop=(k == NKD - 1),
                            )
                    return go

                def evict():
                    for ps, bias, dstT in zip(pss, (bq_sb, bk_sb), (QT, KT)):
                        nc.vector.tensor_scalar_add(
                            dstT[mi][ni][:, :], ps[:], bias[:, mi : mi + 1]
                        )

                return [mk(k) for k in range(NKD)] + [evict]

            vp = {}

            def emit_transposes(pair, i_lo, i_hi):
                # V' tiles [128, 132] via XBAR dma transpose from QT:
                # A data 0:64, A one 64, B data 66:130, B one 130.
                for i in range(i_lo, i_hi):
                    if (pair, i) in vp:
                        continue
                    src = QT[pair][i // 4][:, (i % 4) * P : (i % 4 + 1) * P]
                    # B data sits at col 80: XBAR writes require the out
                    # offset to be 32-byte aligned (16 bf16 cols)
                    vt = vpool.tile([P, 148], BF16, name="vt")
                    nc.sync.dma_start_transpose(out=vt[:, 0:64], in_=src[0:64, :])
                    nc.sync.dma_start_transpose(out=vt[:, 80:144], in_=src[64:128, :])
                    nc.gpsimd.tensor_copy(vt[:, 64:65], ones_col[:])
                    nc.gpsimd.tensor_copy(vt[:, 144:145], ones_col[:])
                    vp[(pair, i)] = vt

            def emit_transposes_pe(pair, i_lo, i_hi):
                # startup variant: PE transposes (PE is idle while DMAs land,
                # and the DMA/HWDGE path is saturated with input loads)
                # shares the "aps" slots: dead before the first atB allocation
                tp = apsum.tile([P, 512], BF16, name="aps")
                for i in range(i_lo, i_hi):
                    c = (i - i_lo) * P
                    nc.tensor.transpose(
                        tp[:, c : c + P],
                        QT[pair][i // 4][:, (i % 4) * P : (i % 4 + 1) * P],
                        identity[:],
                    )
                    vt = vpool.tile([P, 148], BF16, name="vt")
                    nc.vector.tensor_copy(vt[:, 0:64], tp[:, c : c + 64])
                    nc.vector.tensor_copy(vt[:, 80:144], tp[:, c + 64 : c + P])
                    nc.gpsimd.tensor_copy(vt[:, 64:65], ones_col[:])
                    nc.gpsimd.tensor_copy(vt[:, 144:145], ones_col[:])
                    vp[(pair, i)] = vt

            bg = []  # drip queue of (cost, key, thunk)
            carry = [0.0]

            def drip(budget):
                # carry-based pacing: costs are ~213ns PE units; fractional
                # budget accumulates so fill work spreads over the whole
                # attention phase instead of draining the queue early
                carry[0] += budget
                while bg and carry[0] >= bg[0][0]:
                    cost, _key, thunk = bg.pop(0)
                    thunk()
                    carry[0] -= cost

            def ensure(key):
                # need-driven drain: emit every queued item with this key now
                # (deadline reached), regardless of queue position
                rest, hits = [], []
                for item in bg:
                    (hits if item[1] == key else rest).append(item)
                if hits:
                    bg[:] = rest
                    for _cost, _key, thunk in hits:
                        thunk()

            def emit_cblock_m(j, c):
                m = j * 4 + c
                ot = ostp.tile([P, D], BF16, name="ot")
                for n in range(2):
                    ps = mmp.tile([P, 512], F32, name="ps")
                    for kc in range(2):
                        nc.tensor.matmul(
                            ps[:],
                            lhsT=attn[j][kc][:, c * P : (c + 1) * P],
                            rhs=w0_sb[kc][:, n * 512 : (n + 1) * 512],
                            start=(kc == 0),
                            stop=(kc == 1),
                        )
                    osl = slice(n * 512, (n + 1) * 512)
                    # n=0 evicts on ACT to keep DVE clear for the normalize
                    # chain (which gates the next q-chunk's AV psum)
                    if n == 0:
                        nc.scalar.copy(ot[:, osl], ps[:])
                    else:
                        nc.vector.tensor_copy(ot[:, osl], ps[:])
                    nc.sync.dma_start(
                        out=out[m * P : (m + 1) * P, osl], in_=ot[:, osl]
                    )

            def emit_pair(pair):
                steps = [(j, i) for j in range(NQ) for i in range(4 * j + 4)]
                ats = {}
                pend = None

                def emit_S(j, i):
                    off = max(0, i * P - j * 512)  # 0,128,256,384
                    w = 512 - off
                    kc = slice((i % 4) * P, (i % 4 + 1) * P)
                    sAB = spsum.tile([P, 1024], F32, name="sAB")
                    nc.tensor.matmul(
                        sAB[:, off:512],
                        lhsT=KT[pair][i // 4][0:64, kc],
                        rhs=QT[pair][j][0:64, off:512],
                    )
                    nc.tensor.matmul(
                        sAB[:, 512 + off : 1024],
                        lhsT=KT[pair][i // 4][64:128, kc],
                        rhs=QT[pair][j][64:128, off:512],
                    )
                    pAB = ptp.tile([P, 1024], BF16, name="pAB")
                    if w == 512:
                        nc.scalar.activation(out=pAB[:, :], in_=sAB[:, :], func=EXP)
                    else:
                        nc.scalar.activation(
                            out=pAB[:].rearrange("p (t c) -> p t c", t=2)[:, :, off:512],
                            in_=sAB[:].rearrange("p (t c) -> p t c", t=2)[:, :, off:512],
                            func=EXP,
                        )
                    if i >= 4 * j:  # diagonal tile: mask the leading block
                        nc.vector.tensor_mul(
                            pAB[:, off : off + P], pAB[:, off : off + P], tri[:]
                        )
                        nc.vector.tensor_mul(
                            pAB[:, 512 + off : 512 + off + P],
                            pAB[:, 512 + off : 512 + off + P],
                            tri[:],
                        )
                    return (j, i, pAB, off, w)

                def emit_AV(j, i, pAB, off, w):
                    if i == 0:
                        ats[j] = (
                            apsum.tile([P, 512], F32, name="aps"),
                            apsum.tile([P, 512], F32, name="aps"),
                        )
                    atA, atB = ats[j]
                    imax = 4 * j + 3
                    qsl = slice(off, 512)
                    vt = vp[(pair, i)]
                    last = i == imax

                    def norm(at, rows):
                        # attn = att_un / d, d = row 64; the mul is split in
                        # column halves so downstream cblocks can start on the
                        # first half while the second is still writing
                        rec = dpool.tile([1, 512], F32, name="rec")
                        nc.vector.reciprocal(rec[:], at[64:65, :])
                        rbc = dpool.tile([64, 512], F32, name="rbc")
                        nc.gpsimd.partition_broadcast(rbc[0:64, :], rec[0:1, :])
                        for h in range(2):
                            cs = slice(h * 256, (h + 1) * 256)
                            nc.vector.tensor_mul(
                                attn[j][pair][rows, cs], at[0:64, cs], rbc[0:64, cs]
                            )

                    nc.tensor.matmul(
                        atA[0:65, qsl],
                        lhsT=vt[:, 0:65],
                        rhs=pAB[:, off:512],
                        start=(i == 0),
                        stop=last,
                    )
                    if last:  # head A normalizes while head B's AV runs on PE
                        norm(atA, slice(0, 64))
                    nc.tensor.matmul(
                        atB[0:65, qsl],
                        lhsT=vt[:, 80:145],
                        rhs=pAB[:, 512 + off : 1024],
                        start=(i == 0),
                        stop=last,
                    )
                    if last:
                        norm(atB, slice(64, 128))
                        if pair == 1:  # output projection becomes available
                            for c in range(4):
                                bg.append(
                                    (4, ("cb", j, c), lambda j=j, c=c: emit_cblock_m(j, c))
                                )

                for j, i in steps:
                    if i == 0:
                        # deadline: this j's QT/KT sweep must be fully emitted
                        ensure(("sw", j, pair))
                    if (pair, i) not in vp:
                        emit_transposes(pair, i, i + 1)
                    cur = emit_S(j, i)
                    drip(2)
                    if pend is not None:
                        emit_AV(*pend)
                    pend = cur
                emit_AV(*pend)

            def t_item(pair, i):
                return (0, ("t", pair, i), lambda: emit_transposes(pair, i, i + 1))

            def sweep_bg(ni, mi):
                # per-k items are 2 matmuls of 512 cols (~427ns); evict is DVE
                return [
                    (2 if ix < NKD else 0, ("sw", ni, mi), it)
                    for ix, it in enumerate(sweep_items(ni, mi))
                ]

            # upfront: pair-0 ni=0 projection (DMA-paced) + first V transposes
            for it in sweep_items(0, 0):
                it()
            emit_transposes_pe(0, 0, 4)
            # bg order follows need-by and DMA-arrival order
            bg.extend(sweep_bg(1, 0))
            bg.extend(t_item(0, i) for i in range(4, 8))
            bg.extend(sweep_bg(2, 0))
            bg.extend(t_item(0, i) for i in range(8, 12))
            bg.extend(sweep_bg(3, 0))
            bg.extend(t_item(0, i) for i in range(12, 16))
            bg.extend(sweep_bg(0, 1))
            bg.extend(sweep_bg(1, 1))
            bg.extend(sweep_bg(2, 1))
            bg.extend(sweep_bg(3, 1))
            bg.extend(t_item(1, i) for i in range(0, 16))
            emit_pair(0)
            emit_pair(1)
            while bg:
                drip(8)

    nc.compile()
    return nc


def make_in_maps(pos_encode_toks, Wq, bq, Wk, bk, W0, b0):
    x = np.asarray(pos_encode_toks, dtype=np.float32)
    Wq = np.asarray(Wq, dtype=np.float32)
    bq = np.asarray(bq, dtype=np.float32)
    Wk = np.asarray(Wk, dtype=np.float32)
    bk = np.asarray(bk, dtype=np.float32)
    W0 = np.asarray(W0, dtype=np.float32)
    in_maps = []
    for core in range(8):
        b, g = divmod(core, 4)
        hs = slice(g * HD, (g + 1) * HD)
        scale = np.float32(1.0 / np.sqrt(DK))
        in_maps.append(
            {
                "xT": np.ascontiguousarray(x[b].T).astype(ml_dtypes.bfloat16),
                "Wq": np.ascontiguousarray(Wq[:, hs]).astype(ml_dtypes.bfloat16),
                "Wk": np.ascontiguousarray(Wk[:, hs] * scale).astype(ml_dtypes.bfloat16),
                "bqt": np.ascontiguousarray(bq[hs].reshape(2, P).T),
                "bkt": np.ascontiguousarray((bk[hs] * scale).reshape(2, P).T),
                "W0": np.ascontiguousarray(W0[hs, :]).astype(ml_dtypes.bfloat16),
            }
        )
    return in_maps


def assemble(results, b0):
    out = np.zeros((2, S, D), dtype=np.float32)
    for core in range(8):
        b = core // 4
        out[b] += results[core]["out"].astype(np.float32)
    out += np.asarray(b0, dtype=np.float32)
    return out


def kernel(pos_encode_toks, Wq, bq, Wk, bk, W0, b0):
    from concourse.bass_utils import run_bass_kernel_spmd

    global _CACHED_NC
    if _CACHED_NC is None:
        _CACHED_NC = build_nc()
    in_maps = make_in_maps(pos_encode_toks, Wq, bq, Wk, bk, W0, b0)
    res = run_bass_kernel_spmd(_CACHED_NC, in_maps, core_ids=list(range(8)))
    return assemble(res.results, b0)


# revision 35
# speedup vs baseline: 1.1679x; 1.0362x over previous
"""Multi-head causal self-attention (V=Q variant) on 8 Trainium2 cores.

Sharding: batch (2) x head-group (4 groups of 4 heads). Each core computes
full-sequence attention for its 4 heads of one batch element, plus its slice
of the output projection; the host sums the 4 partial projections per batch
and adds b0.

Per core: xT [1024, 2048], Wq_s/Wk_s [1024, 256], W0_s [256, 1024].

Scores are computed transposed (S^T[kv, q]) so the softmax denominator falls
out of the AV matmul via a ones-column appended to V (V aliases Q in this
module -- the reference replicates that bug). The 1/sqrt(DK) scale is folded
into Wk/bk on the host. All matmul operands are bf16 (1 cycle/row on the PE);
PSUM accumulation stays fp32.

The two heads of a pair share one [128,1024] score PSUM tile so a single ACT
exp instruction covers both (halves ACT instruction count). V' tiles are
produced by DMA-engine transposes (XBAR) straight from the bf16 QT tiles,
keeping the PE out of the transpose path and DVE out of the evictions. ACT
(exp) is the attention-phase rate limiter, so program order interleaves
projection sub-sweeps and output-projection blocks into the PE's exp-wait
gaps via a drip queue. Causal diagonal tiles are narrowed to their valid
q-range (left-trimmed).
"""

import ml_dtypes
import numpy as np

import concourse.bacc as bacc
import concourse.mybir as mybir
from concourse.tile import TileContext

P = 128
S = 2048  # sequence length
D = 1024  # model dim
HD = 256  # head-group width (4 heads x 64)
DK = 64
NQ = 4  # q chunks of 512
NKV = 16  # kv chunks of 128
NKD = 8  # D chunks of 128
F32 = mybir.dt.float32
BF16 = mybir.dt.bfloat16
EXP = mybir.ActivationFunctionType.Exp

_CACHED_NC = None


def build_nc():
    nc = bacc.Bacc("TRN2", target_bir_lowering=False, debug=False, num_devices=8)
    xT = nc.declare_dram_parameter("xT", [D, S], BF16, isOutput=False)
    Wq = nc.declare_dram_parameter("Wq", [D, HD], BF16, isOutput=False)
    Wk = nc.declare_dram_parameter("Wk", [D, HD], BF16, isOutput=False)
    bqt = nc.declare_dram_parameter("bqt", [P, 2], F32, isOutput=False)
    bkt = nc.declare_dram_parameter("bkt", [P, 2], F32, isOutput=False)
    W0 = nc.declare_dram_parameter("W0", [HD, D], BF16, isOutput=False)
    out = nc.declare_dram_parameter("out", [S, D], BF16, isOutput=True)

    with TileContext(nc) as tc:
        with (
            tc.tile_pool(name="const", bufs=1) as const,
            tc.tile_pool(name="xt", bufs=16) as xtp,
            tc.tile_pool(name="wqk", bufs=1) as wp,
            tc.tile_pool(name="vp", bufs=32) as vpool,
            tc.tile_pool(name="pt", bufs=6) as ptp,
            tc.tile_pool(name="dp", bufs=4) as dpool,
            tc.tile_pool(name="ost", bufs=3) as ostp,
            tc.tile_pool(name="mm", bufs=2, space="PSUM") as mmp,
            tc.tile_pool(name="sps", bufs=2, space="PSUM") as spsum,
            tc.tile_pool(name="aps", bufs=2, space="PSUM") as apsum,
        ):
            ones_col = const.tile([P, 1], BF16)
            nc.gpsimd.memset(ones_col[:], 1.0)
            identity = const.tile([P, P], BF16)
            nc.gpsimd.memset(identity[:], 0.0)
            nc.gpsimd.affine_select(
                out=identity[:],
                in_=identity[:],
                compare_op=mybir.AluOpType.not_equal,
                fill=1.0,
                base=0,
                pattern=[[-1, P]],
                channel_multiplier=1,
            )
            # triangular mask [128,128] bf16: keep (1.0) where q >= kv
            tri = const.tile([P, P], BF16, name="tri")
            nc.gpsimd.memset(tri[:], 1.0)
            nc.gpsimd.affine_select(
                out=tri[:],
                in_=tri[:],
                compare_op=mybir.AluOpType.is_ge,
                fill=0.0,
                base=0,
                pattern=[[1, P]],
                channel_multiplier=-1,
            )
            # ACT exp-table warmup while DMAs run
            warm = const.tile([P, 8], F32, name="warm")
            nc.gpsimd.memset(warm[:], 0.0)
            nc.scalar.activation(out=warm[:], in_=warm[:], func=EXP)

            bq_sb = const.tile([P, 2], F32)
            bk_sb = const.tile([P, 2], F32)
            w0_sb = [const.tile([P, D], BF16, name=f"w0_{kc}") for kc in range(2)]
            # QT/KT as [mi][ni] tiles of [128, 512] for fine-grained deps
            QT = [
                [const.tile([P, 512], BF16, name=f"qt{mi}_{ni}") for ni in range(NQ)]
                for mi in range(2)
            ]
            KT = [
                [const.tile([P, 512], BF16, name=f"kt{mi}_{ni}") for ni in range(NQ)]
                for mi in range(2)
            ]
            # normalized attention (transposed), per q-chunk and head-pair
            attn = [
                [const.tile([P, 512], BF16, name=f"attn{j}_{p}") for p in range(2)]
                for j in range(4)
            ]

            # first k-slice of each weight as its own tile/DMA so the first
            # projection sub-sweep starts as soon as ~130KB has landed
            wq_t0 = wp.tile([P, HD], BF16, name="wq0")
            wk_t0 = wp.tile([P, HD], BF16, name="wk0")
            wq_r = wp.tile([P, NKD - 1, HD], BF16, name="wqr")
            wk_r = wp.tile([P, NKD - 1, HD], BF16, name="wkr")
            wq_view = Wq.rearrange("(k p) c -> p k c", p=P)
            wk_view = Wk.rearrange("(k p) c -> p k c", p=P)
            nc.sync.dma_start(out=wq_t0[:], in_=wq_view[:, 0, :])
            xh = [
                [xtp.tile([P, 1024], BF16, name="xtile") for _ in range(2)]
                for _ in range(NKD)
            ]
            nc.sync.dma_start(out=xh[0][0][:], in_=xT[0:P, 0:1024])
            nc.sync.dma_start(out=wk_t0[:], in_=wk_view[:, 0, :])
            nc.sync.dma_start(out=bq_sb[:], in_=bqt[:, :])
            nc.sync.dma_start(out=bk_sb[:], in_=bkt[:, :])
            nc.sync.dma_start(out=wq_r[:], in_=wq_view[:, 1:NKD, :])
            nc.sync.dma_start(out=wk_r[:], in_=wk_view[:, 1:NKD, :])
            wq_t = [wq_t0[:]] + [wq_r[:, k - 1, :] for k in range(1, NKD)]
            wk_t = [wk_t0[:]] + [wk_r[:, k - 1, :] for k in range(1, NKD)]
            # x half-0 chunks first, then half-1, then W0: the DMA path
            # drains in issue order, which staggers arrivals naturally
            for h in range(2):
                for k in range(NKD):
                    if h == 0 and k == 0:
                        continue
                    nc.sync.dma_start(
                        out=xh[k][h][:],
                        in_=xT[k * P : (k + 1) * P, h * 1024 : (h + 1) * 1024],
                    )
            for kc in range(2):
                nc.sync.dma_start(out=w0_sb[kc][:], in_=W0[kc * P : (kc + 1) * P, :])

            def sweep_items(ni, mi):
                """Projection sub-sweep as a list of emit-thunks (per-k)."""
                half, col = divmod(ni, 2)
                pss = [mmp.tile([P, 512], F32, name="ps") for _ in range(2)]

                def mk(k):
                    def go():
                        for ps, wt in zip(pss, (wq_t, wk_t)):
                            nc.tensor.matmul(
                                ps[:],
                                lhsT=wt[k][:, mi * P : (mi + 1) * P],
                                rhs=xh[k][half][:, col * 512 : (col + 1) * 512],
                                start=(k == 0),
                                stop=(k == NKD - 1),
                            )
                    return go

                def evict():
                    for ps, bias, dstT in zip(pss, (bq_sb, bk_sb), (QT, KT)):
                        nc.vector.tensor_scalar_add(
                            dstT[mi][ni][:, :], ps[:], bias[:, mi : mi + 1]
                        )

                return [mk(k) for k in range(NKD)] + [evict]

            vp = {}

            def emit_transposes(pair, i_lo, i_hi):
                # V' tiles [128, 132] via XBAR dma transpose from QT:
                # A data 0:64, A one 64, B data 66:130, B one 130.
                for i in range(i_lo, i_hi):
                    if (pair, i) in vp:
                        continue
                    src = QT[pair][i // 4][:, (i % 4) * P : (i % 4 + 1) * P]
                    # B data sits at col 80: XBAR writes require the out
                    # offset to be 32-byte aligned (16 bf16 cols)
                    vt = vpool.tile([P, 148], BF16, name="vt")
                    nc.sync.dma_start_transpose(out=vt[:, 0:64], in_=src[0:64, :])
                    nc.sync.dma_start_transpose(out=vt[:, 80:144], in_=src[64:128, :])
                    nc.gpsimd.tensor_copy(vt[:, 64:65], ones_col[:])
                    nc.gpsimd.tensor_copy(vt[:, 144:145], ones_col[:])
                    vp[(pair, i)] = vt

            def emit_transposes_pe(pair, i_lo, i_hi):
                # startup variant: PE transposes (PE is idle while DMAs land,
                # and the DMA/HWDGE path is saturated with input loads)
                # shares the "aps" slots: dead before the first atB allocation
                tp = apsum.tile([P, 512], BF16, name="aps")
                for i in range(i_lo, i_hi):
                    c = (i - i_lo) * P
                    nc.tensor.transpose(
                        tp[:, c : c + P],
                        QT[pair][i // 4][:, (i % 4) * P : (i % 4 + 1) * P],
                        identity[:],
                    )
                    vt = vpool.tile([P, 148], BF16, name="vt")
                    nc.vector.tensor_copy(vt[:, 0:64], tp[:, c : c + 64])
                    nc.vector.tensor_copy(vt[:, 80:144], tp[:, c + 64 : c + P])
                    nc.gpsimd.tensor_copy(vt[:, 64:65], ones_col[:])
                    nc.gpsimd.tensor_copy(vt[:, 144:145], ones_col[:])
                    vp[(pair, i)] = vt

            bg = []  # drip queue of (cost, key, thunk)
            carry = [0.0]

            def drip(budget):
                # carry-based pacing: costs are ~213ns PE units; fractional
                # budget accumulates so fill work spreads over the whole
                # attention phase instead of draining the queue early
                carry[0] += budget
                while bg and carry[0] >= bg[0][0]:
                    cost, _key, thunk = bg.pop(0)
                    thunk()
                    carry[0] -= cost

            def ensure(key):
                # need-driven drain: emit every queued item with this key now
                # (deadline reached), regardless of queue position
                rest, hits = [], []
                for item in bg:
                    (hits if item[1] == key else rest).append(item)
                if hits:
                    bg[:] = rest
                    for _cost, _key, thunk in hits:
                        thunk()

            def emit_cblock_m(j, c, act_only=False):
                m = j * 4 + c
                ot = ostp.tile([P, D], BF16, name="ot")
                for n in range(2):
                    ps = mmp.tile([P, 512], F32, name="ps")
                    for kc in range(2):
                        nc.tensor.matmul(
                            ps[:],
                            lhsT=attn[j][kc][:, c * P : (c + 1) * P],
                            rhs=w0_sb[kc][:, n * 512 : (n + 1) * 512],
                            start=(kc == 0),
                            stop=(kc == 1),
                        )
                    osl = slice(n * 512, (n + 1) * 512)
                    # n=0 evicts on ACT to keep DVE clear for the normalize
                    # chain (which gates the next q-chunk's AV psum); in the
                    # endgame (act_only) ACT takes both halves since the DVE
                    # is the pacing engine there
                    if n == 0 or act_only:
                        nc.scalar.copy(ot[:, osl], ps[:])
                    else:
                        nc.vector.tensor_copy(ot[:, osl], ps[:])
                nc.sync.dma_start(out=out[m * P : (m + 1) * P, :], in_=ot[:])

            def emit_pair(pair):
                steps = [(j, i) for j in range(NQ) for i in range(4 * j + 4)]
                ats = {}

                def emit_S(j, i):
                    off = max(0, i * P - j * 512)  # 0,128,256,384
                    w = 512 - off
                    kc = slice((i % 4) * P, (i % 4 + 1) * P)
                    sAB = spsum.tile([P, 1024], F32, name="sAB")
                    nc.tensor.matmul(
                        sAB[:, off:512],
                        lhsT=KT[pair][i // 4][0:64, kc],
                        rhs=QT[pair][j][0:64, off:512],
                    )
                    nc.tensor.matmul(
                        sAB[:, 512 + off : 1024],
                        lhsT=KT[pair][i // 4][64:128, kc],
                        rhs=QT[pair][j][64:128, off:512],
                    )
                    pAB = ptp.tile([P, 1024], BF16, name="pAB")
                    if w == 512:
                        nc.scalar.activation(out=pAB[:, :], in_=sAB[:, :], func=EXP)
                    else:
                        nc.scalar.activation(
                            out=pAB[:].rearrange("p (t c) -> p t c", t=2)[:, :, off:512],
                            in_=sAB[:].rearrange("p (t c) -> p t c", t=2)[:, :, off:512],
                            func=EXP,
                        )
                    if i >= 4 * j:  # diagonal tile: mask the leading block
                        nc.vector.tensor_mul(
                            pAB[:, off : off + P], pAB[:, off : off + P], tri[:]
                        )
                        nc.vector.tensor_mul(
                            pAB[:, 512 + off : 512 + off + P],
                            pAB[:, 512 + off : 512 + off + P],
                            tri[:],
                        )
                    return (j, i, pAB, off, w)

                def emit_AV(j, i, pAB, off, w):
                    if i == 0:
                        ats[j] = (
                            apsum.tile([P, 512], F32, name="aps"),
                            apsum.tile([P, 512], F32, name="aps"),
                        )
                    atA, atB = ats[j]
                    imax = 4 * j + 3
                    qsl = slice(off, 512)
                    vt = vp[(pair, i)]
                    last = i == imax

                    nc.tensor.matmul(
                        atA[0:65, qsl],
                        lhsT=vt[:, 0:65],
                        rhs=pAB[:, off:512],
                        start=(i == 0),
                        stop=last,
                    )
                    final = pair == 1 and j == NQ - 1
                    recA = None
                    if last and not final:
                        # head A's reciprocal runs while B's AV is on PE
                        recA = dpool.tile([1, 512], F32, name="rec")
                        nc.vector.reciprocal(recA[:], atA[64:65, :])
                    nc.tensor.matmul(
                        atB[0:65, qsl],
                        lhsT=vt[:, 80:145],
                        rhs=pAB[:, 512 + off : 1024],
                        start=(i == 0),
                        stop=last,
                    )
                    if last:
                        if final:
                            # final tile: the whole normalize chain runs at
                            # column-quarter granularity, each cblock emitted
                            # right after the quarter it consumes, so the
                            # first cblock starts ~1us after the last AV
                            rA = dpool.tile([1, 512], F32, name="rec")
                            rB = dpool.tile([1, 512], F32, name="rec")
                            bA = dpool.tile([64, 512], F32, name="rbc")
                            bB = dpool.tile([64, 512], F32, name="rbc")
                            for c in range(4):
                                cs = slice(c * P, (c + 1) * P)
                                nc.vector.reciprocal(rA[:, cs], atA[64:65, cs])
                                nc.vector.reciprocal(rB[:, cs], atB[64:65, cs])
                                nc.gpsimd.partition_broadcast(bA[0:64, cs], rA[0:1, cs])
                                nc.gpsimd.partition_broadcast(bB[0:64, cs], rB[0:1, cs])
                                nc.vector.tensor_mul(
                                    attn[j][pair][0:64, cs], atA[0:64, cs], bA[0:64, cs]
                                )
                                nc.vector.tensor_mul(
                                    attn[j][pair][64:128, cs], atB[0:64, cs], bB[0:64, cs]
                                )
                                emit_cblock_m(j, c, act_only=True)
                            return
                        # normalize attn = att_un / d (d = psum row 64), with
                        # the A and B chains pipelined across DVE and Pool and
                        # the muls in column halves so cblocks unblock early
                        recB = dpool.tile([1, 512], F32, name="rec")
                        nc.vector.reciprocal(recB[:], atB[64:65, :])
                        rbcA = dpool.tile([64, 512], F32, name="rbc")
                        nc.gpsimd.partition_broadcast(rbcA[0:64, :], recA[0:1, :])
                        rbcB = dpool.tile([64, 512], F32, name="rbc")
                        nc.gpsimd.partition_broadcast(rbcB[0:64, :], recB[0:1, :])
                        for h in range(2):
                            cs = slice(h * 256, (h + 1) * 256)
                            nc.vector.tensor_mul(
                                attn[j][pair][0:64, cs], atA[0:64, cs], rbcA[0:64, cs]
                            )
                            nc.vector.tensor_mul(
                                attn[j][pair][64:128, cs], atB[0:64, cs], rbcB[0:64, cs]
                            )
                        if pair == 1:  # output projection becomes available
                            for c in range(4):
                                bg.append(
                                    (4, ("cb", j, c), lambda j=j, c=c: emit_cblock_m(j, c))
                                )

                pend = []
                for j, i in steps:
                    if i == 0:
                        # deadline: this j's QT/KT sweep must be fully emitted
                        ensure(("sw", j, pair))
                    if (pair, i) not in vp:
                        emit_transposes(pair, i, i + 1)
                    pend.append(emit_S(j, i))
                    drip(3 if i <= 2 else 1.5)
                    # depth-2 software pipeline: AV(T) trails S(T) by two
                    # tiles, so the exp on ACT has two tile-periods of slack
                    if len(pend) > 3:
                        emit_AV(*pend.pop(0))
                for p_ in pend:
                    emit_AV(*p_)

            def t_item(pair, i):
                return (0, ("t", pair, i), lambda: emit_transposes(pair, i, i + 1))

            def sweep_bg(ni, mi):
                # per-k items are 2 matmuls of 512 cols (~427ns); evict is DVE
                return [
                    (2 if ix < NKD else 0, ("sw", ni, mi), it)
                    for ix, it in enumerate(sweep_items(ni, mi))
                ]

            # upfront: pair-0 ni=0 projection (DMA-paced) + first V transposes
            for it in sweep_items(0, 0):
                it()
            emit_transposes_pe(0, 0, 4)
            # bg order follows need-by and DMA-arrival order
            bg.extend(sweep_bg(1, 0))
            bg.extend(t_item(0, i) for i in range(4, 8))
            bg.extend(sweep_bg(2, 0))
            bg.extend(t_item(0, i) for i in range(8, 12))
            bg.extend(sweep_bg(3, 0))
            bg.extend(t_item(0, i) for i in range(12, 16))
            bg.extend(sweep_bg(0, 1))
            bg.extend(sweep_bg(1, 1))
            bg.extend(sweep_bg(2, 1))
            bg.extend(sweep_bg(3, 1))
            bg.extend(t_item(1, i) for i in range(0, 16))
            emit_pair(0)
            emit_pair(1)
            while bg:
                drip(8)

    nc.compile()
    return nc


def make_in_maps(pos_encode_toks, Wq, bq, Wk, bk, W0, b0):
    x = np.asarray(pos_encode_toks, dtype=np.float32)
    Wq = np.asarray(Wq, dtype=np.float32)
    bq = np.asarray(bq, dtype=np.float32)
    Wk = np.asarray(Wk, dtype=np.float32)
    bk = np.asarray(bk, dtype=np.float32)
    W0 = np.asarray(W0, dtype=np.float32)
    in_maps = []
    for core in range(8):
        b, g = divmod(core, 4)
        hs = slice(g * HD, (g + 1) * HD)
        scale = np.float32(1.0 / np.sqrt(DK))
        in_maps.append(
            {
                "xT": np.ascontiguousarray(x[b].T).astype(ml_dtypes.bfloat16),
                "Wq": np.ascontiguousarray(Wq[:, hs]).astype(ml_dtypes.bfloat16),
                "Wk": np.ascontiguousarray(Wk[:, hs] * scale).astype(ml_dtypes.bfloat16),
                "bqt": np.ascontiguousarray(bq[hs].reshape(2, P).T),
                "bkt": np.ascontiguousarray((bk[hs] * scale).reshape(2, P).T),
                "W0": np.ascontiguousarray(W0[hs, :]).astype(ml_dtypes.bfloat16),
            }
        )
    return in_maps


def assemble(results, b0):
    out = np.zeros((2, S, D), dtype=np.float32)
    for core in range(8):
        b = core // 4
        out[b] += results[core]["out"].astype(np.float32)
    out += np.asarray(b0, dtype=np.float32)
    return out


def kernel(pos_encode_toks, Wq, bq, Wk, bk, W0, b0):
    from concourse.bass_utils import run_bass_kernel_spmd

    global _CACHED_NC
    if _CACHED_NC is None:
        _CACHED_NC = build_nc()
    in_maps = make_in_maps(pos_encode_toks, Wq, bq, Wk, bk, W0, b0)
    res = run_bass_kernel_spmd(_CACHED_NC, in_maps, core_ids=list(range(8)))
    return assemble(res.results, b0)


# revision 43
# speedup vs baseline: 1.1707x; 1.0024x over previous
"""Multi-head causal self-attention (V=Q variant) on 8 Trainium2 cores.

Sharding: batch (2) x head-group (4 groups of 4 heads). Each core computes
full-sequence attention for its 4 heads of one batch element, plus its slice
of the output projection; the host sums the 4 partial projections per batch
and adds b0.

Per core: xT [1024, 2048], Wq_s/Wk_s [1024, 256], W0_s [256, 1024].

Scores are computed transposed (S^T[kv, q]) so the softmax denominator falls
out of the AV matmul via a ones-column appended to V (V aliases Q in this
module -- the reference replicates that bug). The 1/sqrt(DK) scale is folded
into Wk/bk on the host. All matmul operands are bf16 (1 cycle/row on the PE);
PSUM accumulation stays fp32.

The two heads of a pair share one [128,1024] score PSUM tile so a single ACT
exp instruction covers both (halves ACT instruction count). V' tiles are
produced by DMA-engine transposes (XBAR) straight from the bf16 QT tiles,
keeping the PE out of the transpose path and DVE out of the evictions. ACT
(exp) is the attention-phase rate limiter, so program order interleaves
projection sub-sweeps and output-projection blocks into the PE's exp-wait
gaps via a drip queue. Causal diagonal tiles are narrowed to their valid
q-range (left-trimmed).
"""

import ml_dtypes
import numpy as np

import concourse.bacc as bacc
import concourse.mybir as mybir
from concourse.tile import TileContext

P = 128
S = 2048  # sequence length
D = 1024  # model dim
HD = 256  # head-group width (4 heads x 64)
DK = 64
NQ = 4  # q chunks of 512
NKV = 16  # kv chunks of 128
NKD = 8  # D chunks of 128
F32 = mybir.dt.float32
BF16 = mybir.dt.bfloat16
EXP = mybir.ActivationFunctionType.Exp

_CACHED_NC = None


def build_nc():
    nc = bacc.Bacc("TRN2", target_bir_lowering=False, debug=False, num_devices=8)
    xT = nc.declare_dram_parameter("xT", [D, S], BF16, isOutput=False)
    Wq = nc.declare_dram_parameter("Wq", [D, HD], BF16, isOutput=False)
    Wk = nc.declare_dram_parameter("Wk", [D, HD], BF16, isOutput=False)
    bqt = nc.declare_dram_parameter("bqt", [P, 2], F32, isOutput=False)
    bkt = nc.declare_dram_parameter("bkt", [P, 2], F32, isOutput=False)
    W0 = nc.declare_dram_parameter("W0", [HD, D], BF16, isOutput=False)
    out = nc.declare_dram_parameter("out", [S, D], BF16, isOutput=True)

    with TileContext(nc) as tc:
        with (
            tc.tile_pool(name="const", bufs=1) as const,
            tc.tile_pool(name="xt", bufs=16) as xtp,
            tc.tile_pool(name="wqk", bufs=1) as wp,
            tc.tile_pool(name="vp", bufs=32) as vpool,
            tc.tile_pool(name="pt", bufs=6) as ptp,
            tc.tile_pool(name="dp", bufs=4) as dpool,
            tc.tile_pool(name="ost", bufs=3) as ostp,
            tc.tile_pool(name="mm", bufs=2, space="PSUM") as mmp,
            tc.tile_pool(name="sps", bufs=2, space="PSUM") as spsum,
            tc.tile_pool(name="aps", bufs=2, space="PSUM") as apsum,
        ):
            ones_col = const.tile([P, 1], BF16)
            nc.gpsimd.memset(ones_col[:], 1.0)
            identity = const.tile([P, P], BF16)
            nc.gpsimd.memset(identity[:], 0.0)
            nc.gpsimd.affine_select(
                out=identity[:],
                in_=identity[:],
                compare_op=mybir.AluOpType.not_equal,
                fill=1.0,
                base=0,
                pattern=[[-1, P]],
                channel_multiplier=1,
            )
            # triangular mask [128,128] bf16: keep (1.0) where q >= kv
            tri = const.tile([P, P], BF16, name="tri")
            nc.gpsimd.memset(tri[:], 1.0)
            nc.gpsimd.affine_select(
                out=tri[:],
                in_=tri[:],
                compare_op=mybir.AluOpType.is_ge,
                fill=0.0,
                base=0,
                pattern=[[1, P]],
                channel_multiplier=-1,
            )
            # ACT exp-table warmup while DMAs run
            warm = const.tile([P, 8], F32, name="warm")
            nc.gpsimd.memset(warm[:], 0.0)
            nc.scalar.activation(out=warm[:], in_=warm[:], func=EXP)

            bq_sb = const.tile([P, 2], F32)
            bk_sb = const.tile([P, 2], F32)
            w0_sb = [const.tile([P, D], BF16, name=f"w0_{kc}") for kc in range(2)]
            # QT/KT as [mi][ni] tiles of [128, 512] for fine-grained deps
            QT = [
                [const.tile([P, 512], BF16, name=f"qt{mi}_{ni}") for ni in range(NQ)]
                for mi in range(2)
            ]
            KT = [
                [const.tile([P, 512], BF16, name=f"kt{mi}_{ni}") for ni in range(NQ)]
                for mi in range(2)
            ]
            # normalized attention (transposed), per q-chunk and head-pair
            attn = [
                [const.tile([P, 512], BF16, name=f"attn{j}_{p}") for p in range(2)]
                for j in range(4)
            ]

            # first k-slice of each weight as its own tile/DMA so the first
            # projection sub-sweep starts as soon as ~130KB has landed
            wq_t0 = wp.tile([P, HD], BF16, name="wq0")
            wk_t0 = wp.tile([P, HD], BF16, name="wk0")
            wq_r = wp.tile([P, NKD - 1, HD], BF16, name="wqr")
            wk_r = wp.tile([P, NKD - 1, HD], BF16, name="wkr")
            wq_view = Wq.rearrange("(k p) c -> p k c", p=P)
            wk_view = Wk.rearrange("(k p) c -> p k c", p=P)
            nc.sync.dma_start(out=wq_t0[:], in_=wq_view[:, 0, :])
            xh = [
                [xtp.tile([P, 1024], BF16, name="xtile") for _ in range(2)]
                for _ in range(NKD)
            ]
            nc.sync.dma_start(out=xh[0][0][:], in_=xT[0:P, 0:1024])
            nc.sync.dma_start(out=wk_t0[:], in_=wk_view[:, 0, :])
            nc.sync.dma_start(out=bq_sb[:], in_=bqt[:, :])
            nc.sync.dma_start(out=bk_sb[:], in_=bkt[:, :])
            nc.sync.dma_start(out=wq_r[:], in_=wq_view[:, 1:NKD, :])
            nc.sync.dma_start(out=wk_r[:], in_=wk_view[:, 1:NKD, :])
            wq_t = [wq_t0[:]] + [wq_r[:, k - 1, :] for k in range(1, NKD)]
            wk_t = [wk_t0[:]] + [wk_r[:, k - 1, :] for k in range(1, NKD)]
            # x half-0 chunks first, then half-1, then W0: the DMA path
            # drains in issue order, which staggers arrivals naturally
            for h in range(2):
                for k in range(NKD):
                    if h == 0 and k == 0:
                        continue
                    nc.sync.dma_start(
                        out=xh[k][h][:],
                        in_=xT[k * P : (k + 1) * P, h * 1024 : (h + 1) * 1024],
                    )
            for kc in range(2):
                nc.sync.dma_start(out=w0_sb[kc][:], in_=W0[kc * P : (kc + 1) * P, :])

            def sweep_items(ni, mi):
                """Projection sub-sweep as a list of emit-thunks (per-k)."""
                half, col = divmod(ni, 2)
                pss = [mmp.tile([P, 512], F32, name="ps") for _ in range(2)]

                def mk(k):
                    def go():
                        for ps, wt in zip(pss, (wq_t, wk_t)):
                            nc.tensor.matmul(
                                ps[:],
                                lhsT=wt[k][:, mi * P : (mi + 1) * P],
                                rhs=xh[k][half][:, col * 512 : (col + 1) * 512],
                                start=(k == 0),
                                stop=(k == NKD - 1),
                            )
                    return go

                def evict():
                    for ps, bias, dstT in zip(pss, (bq_sb, bk_sb), (QT, KT)):
                        nc.vector.tensor_scalar_add(
                            dstT[mi][ni][:, :], ps[:], bias[:, mi : mi + 1]
                        )

                return [mk(k) for k in range(NKD)] + [evict]

            vp = {}

            def emit_transposes(pair, i_lo, i_hi):
                # V' tiles [128, 132] via XBAR dma transpose from QT:
                # A data 0:64, A one 64, B data 66:130, B one 130.
                for i in range(i_lo, i_hi):
                    if (pair, i) in vp:
                        continue
                    src = QT[pair][i // 4][:, (i % 4) * P : (i % 4 + 1) * P]
                    # B data sits at col 80: XBAR writes require the out
                    # offset to be 32-byte aligned (16 bf16 cols)
                    vt = vpool.tile([P, 148], BF16, name="vt")
                    nc.sync.dma_start_transpose(out=vt[:, 0:64], in_=src[0:64, :])
                    nc.sync.dma_start_transpose(out=vt[:, 80:144], in_=src[64:128, :])
                    nc.gpsimd.tensor_copy(vt[:, 64:65], ones_col[:])
                    nc.gpsimd.tensor_copy(vt[:, 144:145], ones_col[:])
                    vp[(pair, i)] = vt

            def emit_transposes_pe(pair, i_lo, i_hi):
                # startup variant: PE transposes (PE is idle while DMAs land,
                # and the DMA/HWDGE path is saturated with input loads)
                # shares the "aps" slots: dead before the first atB allocation
                tp = apsum.tile([P, 512], BF16, name="aps")
                for i in range(i_lo, i_hi):
                    c = (i - i_lo) * P
                    nc.tensor.transpose(
                        tp[:, c : c + P],
                        QT[pair][i // 4][:, (i % 4) * P : (i % 4 + 1) * P],
                        identity[:],
                    )
                    vt = vpool.tile([P, 148], BF16, name="vt")
                    nc.vector.tensor_copy(vt[:, 0:64], tp[:, c : c + 64])
                    nc.vector.tensor_copy(vt[:, 80:144], tp[:, c + 64 : c + P])
                    nc.gpsimd.tensor_copy(vt[:, 64:65], ones_col[:])
                    nc.gpsimd.tensor_copy(vt[:, 144:145], ones_col[:])
                    vp[(pair, i)] = vt

            bg = []  # drip queue of (cost, key, thunk)
            carry = [0.0]

            def drip(budget):
                # carry-based pacing: costs are ~213ns PE units; fractional
                # budget accumulates so fill work spreads over the whole
                # attention phase instead of draining the queue early
                carry[0] += budget
                while bg and carry[0] >= bg[0][0]:
                    cost, _key, thunk = bg.pop(0)
                    thunk()
                    carry[0] -= cost

            def ensure(key):
                # need-driven drain: emit every queued item with this key now
                # (deadline reached), regardless of queue position
                rest, hits = [], []
                for item in bg:
                    (hits if item[1] == key else rest).append(item)
                if hits:
                    bg[:] = rest
                    for _cost, _key, thunk in hits:
                        thunk()

            def emit_cblock_m(j, c, act_only=False):
                m = j * 4 + c
                ot = ostp.tile([P, D], BF16, name="ot")
                for n in range(2):
                    ps = mmp.tile([P, 512], F32, name="ps")
                    for kc in range(2):
                        nc.tensor.matmul(
                            ps[:],
                            lhsT=attn[j][kc][:, c * P : (c + 1) * P],
                            rhs=w0_sb[kc][:, n * 512 : (n + 1) * 512],
                            start=(kc == 0),
                            stop=(kc == 1),
                        )
                    osl = slice(n * 512, (n + 1) * 512)
                    # n=0 evicts on ACT to keep DVE clear for the normalize
                    # chain (which gates the next q-chunk's AV psum); in the
                    # endgame (act_only) ACT takes both halves since the DVE
                    # is the pacing engine there
                    if n == 0 or act_only:
                        nc.scalar.copy(ot[:, osl], ps[:])
                    else:
                        nc.vector.tensor_copy(ot[:, osl], ps[:])
                    if act_only:
                        # endgame: per-half DMA so the n=0 half transfers
                        # while the n=1 half is still evicting
                        nc.sync.dma_start(
                            out=out[m * P : (m + 1) * P, osl], in_=ot[:, osl]
                        )
                if not act_only:
                    nc.sync.dma_start(out=out[m * P : (m + 1) * P, :], in_=ot[:])

            def emit_pair(pair):
                steps = [(j, i) for j in range(NQ) for i in range(4 * j + 4)]
                ats = {}

                def emit_S(j, i):
                    off = max(0, i * P - j * 512)  # 0,128,256,384
                    w = 512 - off
                    kc = slice((i % 4) * P, (i % 4 + 1) * P)
                    sAB = spsum.tile([P, 1024], F32, name="sAB")
                    nc.tensor.matmul(
                        sAB[:, off:512],
                        lhsT=KT[pair][i // 4][0:64, kc],
                        rhs=QT[pair][j][0:64, off:512],
                    )
                    nc.tensor.matmul(
                        sAB[:, 512 + off : 1024],
                        lhsT=KT[pair][i // 4][64:128, kc],
                        rhs=QT[pair][j][64:128, off:512],
                    )
                    pAB = ptp.tile([P, 1024], BF16, name="pAB")
                    if w == 512:
                        nc.scalar.activation(out=pAB[:, :], in_=sAB[:, :], func=EXP)
                    else:
                        nc.scalar.activation(
                            out=pAB[:].rearrange("p (t c) -> p t c", t=2)[:, :, off:512],
                            in_=sAB[:].rearrange("p (t c) -> p t c", t=2)[:, :, off:512],
                            func=EXP,
                        )
                    if i >= 4 * j:  # diagonal tile: mask the leading block
                        nc.vector.tensor_mul(
                            pAB[:, off : off + P], pAB[:, off : off + P], tri[:]
                        )
                        nc.vector.tensor_mul(
                            pAB[:, 512 + off : 512 + off + P],
                            pAB[:, 512 + off : 512 + off + P],
                            tri[:],
                        )
                    return (j, i, pAB, off, w)

                def emit_AV(j, i, pAB, off, w):
                    if i == 0:
                        ats[j] = (
                            apsum.tile([P, 512], F32, name="aps"),
                            apsum.tile([P, 512], F32, name="aps"),
                        )
                    atA, atB = ats[j]
                    imax = 4 * j + 3
                    qsl = slice(off, 512)
                    vt = vp[(pair, i)]
                    last = i == imax

                    nc.tensor.matmul(
                        atA[0:65, qsl],
                        lhsT=vt[:, 0:65],
                        rhs=pAB[:, off:512],
                        start=(i == 0),
                        stop=last,
                    )
                    final = pair == 1 and j == NQ - 1
                    recA = None
                    if last and not final:
                        # head A's reciprocal runs while B's AV is on PE
                        recA = dpool.tile([1, 512], F32, name="rec")
                        nc.vector.reciprocal(recA[:], atA[64:65, :])
                    nc.tensor.matmul(
                        atB[0:65, qsl],
                        lhsT=vt[:, 80:145],
                        rhs=pAB[:, 512 + off : 1024],
                        start=(i == 0),
                        stop=last,
                    )
                    if last:
                        if final:
                            # final tile: the whole normalize chain runs at
                            # column-quarter granularity, each cblock emitted
                            # right after the quarter it consumes, so the
                            # first cblock starts ~1us after the last AV
                            rA = dpool.tile([1, 512], F32, name="rec")
                            rB = dpool.tile([1, 512], F32, name="rec")
                            bA = dpool.tile([64, 512], F32, name="rbc")
                            bB = dpool.tile([64, 512], F32, name="rbc")
                            for c in range(4):
                                cs = slice(c * P, (c + 1) * P)
                                nc.vector.reciprocal(rA[:, cs], atA[64:65, cs])
                                nc.vector.reciprocal(rB[:, cs], atB[64:65, cs])
                                nc.gpsimd.partition_broadcast(bA[0:64, cs], rA[0:1, cs])
                                nc.gpsimd.partition_broadcast(bB[0:64, cs], rB[0:1, cs])
                                nc.vector.tensor_mul(
                                    attn[j][pair][0:64, cs], atA[0:64, cs], bA[0:64, cs]
                                )
                                nc.vector.tensor_mul(
                                    attn[j][pair][64:128, cs], atB[0:64, cs], bB[0:64, cs]
                                )
                                emit_cblock_m(j, c, act_only=True)
                            return
                        # normalize attn = att_un / d (d = psum row 64), with
                        # the A and B chains pipelined across DVE and Pool and
                        # the muls in column halves so cblocks unblock early
                        recB = dpool.tile([1, 512], F32, name="rec")
                        nc.vector.reciprocal(recB[:], atB[64:65, :])
                        rbcA = dpool.tile([64, 512], F32, name="rbc")
                        nc.gpsimd.partition_broadcast(rbcA[0:64, :], recA[0:1, :])
                        rbcB = dpool.tile([64, 512], F32, name="rbc")
                        nc.gpsimd.partition_broadcast(rbcB[0:64, :], recB[0:1, :])
                        for h in range(2):
                            cs = slice(h * 256, (h + 1) * 256)
                            nc.vector.tensor_mul(
                                attn[j][pair][0:64, cs], atA[0:64, cs], rbcA[0:64, cs]
                            )
                            nc.vector.tensor_mul(
                                attn[j][pair][64:128, cs], atB[0:64, cs], rbcB[0:64, cs]
                            )
                        if pair == 1:  # output projection becomes available
                            for c in range(4):
                                bg.append(
                                    (4, ("cb", j, c), lambda j=j, c=c: emit_cblock_m(j, c))
                                )

                pend = []
                for j, i in steps:
                    if i == 0:
                        # deadline: this j's QT/KT sweep must be fully emitted
                        ensure(("sw", j, pair))
                    if (pair, i) not in vp:
                        emit_transposes(pair, i, i + 1)
                    pend.append(emit_S(j, i))
                    drip(3 if i <= 2 else 1.5)
                    # depth-2 software pipeline: AV(T) trails S(T) by two
                    # tiles, so the exp on ACT has two tile-periods of slack
                    if len(pend) > 3:
                        emit_AV(*pend.pop(0))
                for p_ in pend:
                    emit_AV(*p_)

            def t_item(pair, i):
                return (0, ("t", pair, i), lambda: emit_transposes(pair, i, i + 1))

            def sweep_bg(ni, mi):
                # per-k items are 2 matmuls of 512 cols (~427ns); evict is DVE
                return [
                    (2 if ix < NKD else 0, ("sw", ni, mi), it)
                    for ix, it in enumerate(sweep_items(ni, mi))
                ]

            # upfront: pair-0 ni=0 projection (DMA-paced) + first V transposes
            for it in sweep_items(0, 0):
                it()
            emit_transposes_pe(0, 0, 4)
            # bg order follows need-by and DMA-arrival order
            bg.extend(sweep_bg(1, 0))
            bg.extend(t_item(0, i) for i in range(4, 8))
            bg.extend(sweep_bg(2, 0))
            bg.extend(t_item(0, i) for i in range(8, 12))
            bg.extend(sweep_bg(3, 0))
            bg.extend(t_item(0, i) for i in range(12, 16))
            bg.extend(sweep_bg(0, 1))
            bg.extend(sweep_bg(1, 1))
            bg.extend(sweep_bg(2, 1))
            bg.extend(sweep_bg(3, 1))
            bg.extend(t_item(1, i) for i in range(0, 16))
            emit_pair(0)
            emit_pair(1)
            while bg:
                drip(8)

    nc.compile()
    return nc


def make_in_maps(pos_encode_toks, Wq, bq, Wk, bk, W0, b0):
    x = np.asarray(pos_encode_toks, dtype=np.float32)
    Wq = np.asarray(Wq, dtype=np.float32)
    bq = np.asarray(bq, dtype=np.float32)
    Wk = np.asarray(Wk, dtype=np.float32)
    bk = np.asarray(bk, dtype=np.float32)
    W0 = np.asarray(W0, dtype=np.float32)
    in_maps = []
    for core in range(8):
        b, g = divmod(core, 4)
        hs = slice(g * HD, (g + 1) * HD)
        scale = np.float32(1.0 / np.sqrt(DK))
        in_maps.append(
            {
                "xT": np.ascontiguousarray(x[b].T).astype(ml_dtypes.bfloat16),
                "Wq": np.ascontiguousarray(Wq[:, hs]).astype(ml_dtypes.bfloat16),
                "Wk": np.ascontiguousarray(Wk[:, hs] * scale).astype(ml_dtypes.bfloat16),
                "bqt": np.ascontiguousarray(bq[hs].reshape(2, P).T),
                "bkt": np.ascontiguousarray((bk[hs] * scale).reshape(2, P).T),
                "W0": np.ascontiguousarray(W0[hs, :]).astype(ml_dtypes.bfloat16),
            }
        )
    return in_maps


def assemble(results, b0):
    out = np.zeros((2, S, D), dtype=np.float32)
    for core in range(8):
        b = core // 4
        out[b] += results[core]["out"].astype(np.float32)
    out += np.asarray(b0, dtype=np.float32)
    return out


def kernel(pos_encode_toks, Wq, bq, Wk, bk, W0, b0):
    from concourse.bass_utils import run_bass_kernel_spmd

    global _CACHED_NC
    if _CACHED_NC is None:
        _CACHED_NC = build_nc()
    in_maps = make_in_maps(pos_encode_toks, Wq, bq, Wk, bk, W0, b0)
    res = run_bass_kernel_spmd(_CACHED_NC, in_maps, core_ids=list(range(8)))
    return assemble(res.results, b0)


# revision 46
# speedup vs baseline: 1.1715x; 1.0007x over previous
"""Multi-head causal self-attention (V=Q variant) on 8 Trainium2 cores.

Sharding: batch (2) x head-group (4 groups of 4 heads). Each core computes
full-sequence attention for its 4 heads of one batch element, plus its slice
of the output projection; the host sums the 4 partial projections per batch
and adds b0.

Per core: xT [1024, 2048], Wq_s/Wk_s [1024, 256], W0_s [256, 1024].

Scores are computed transposed (S^T[kv, q]) so the softmax denominator falls
out of the AV matmul via a ones-column appended to V (V aliases Q in this
module -- the reference replicates that bug). The 1/sqrt(DK) scale is folded
into Wk/bk on the host. All matmul operands are bf16 (1 cycle/row on the PE);
PSUM accumulation stays fp32.

The two heads of a pair share one [128,1024] score PSUM tile so a single ACT
exp instruction covers both (halves ACT instruction count). V' tiles are
produced by DMA-engine transposes (XBAR) straight from the bf16 QT tiles,
keeping the PE out of the transpose path and DVE out of the evictions. ACT
(exp) is the attention-phase rate limiter, so program order interleaves
projection sub-sweeps and output-projection blocks into the PE's exp-wait
gaps via a drip queue. Causal diagonal tiles are narrowed to their valid
q-range (left-trimmed).
"""

import ml_dtypes
import numpy as np

import concourse.bacc as bacc
import concourse.mybir as mybir
from concourse.tile import TileContext

P = 128
S = 2048  # sequence length
D = 1024  # model dim
HD = 256  # head-group width (4 heads x 64)
DK = 64
NQ = 4  # q chunks of 512
NKV = 16  # kv chunks of 128
NKD = 8  # D chunks of 128
F32 = mybir.dt.float32
BF16 = mybir.dt.bfloat16
EXP = mybir.ActivationFunctionType.Exp

_CACHED_NC = None


def build_nc():
    nc = bacc.Bacc("TRN2", target_bir_lowering=False, debug=False, num_devices=8)
    xT = nc.declare_dram_parameter("xT", [D, S], BF16, isOutput=False)
    Wq = nc.declare_dram_parameter("Wq", [D, HD], BF16, isOutput=False)
    Wk = nc.declare_dram_parameter("Wk", [D, HD], BF16, isOutput=False)
    bqt = nc.declare_dram_parameter("bqt", [P, 2], F32, isOutput=False)
    bkt = nc.declare_dram_parameter("bkt", [P, 2], F32, isOutput=False)
    W0 = nc.declare_dram_parameter("W0", [HD, D], BF16, isOutput=False)
    out = nc.declare_dram_parameter("out", [S, D], BF16, isOutput=True)

    with TileContext(nc) as tc:
        with (
            tc.tile_pool(name="const", bufs=1) as const,
            tc.tile_pool(name="xt", bufs=16) as xtp,
            tc.tile_pool(name="wqk", bufs=1) as wp,
            tc.tile_pool(name="vp", bufs=32) as vpool,
            tc.tile_pool(name="pt", bufs=6) as ptp,
            tc.tile_pool(name="dp", bufs=6) as dpool,
            tc.tile_pool(name="ost", bufs=5) as ostp,
            tc.tile_pool(name="mm", bufs=2, space="PSUM") as mmp,
            tc.tile_pool(name="sps", bufs=2, space="PSUM") as spsum,
            tc.tile_pool(name="aps", bufs=2, space="PSUM") as apsum,
        ):
            ones_col = const.tile([P, 1], BF16)
            nc.gpsimd.memset(ones_col[:], 1.0)
            identity = const.tile([P, P], BF16)
            nc.gpsimd.memset(identity[:], 0.0)
            nc.gpsimd.affine_select(
                out=identity[:],
                in_=identity[:],
                compare_op=mybir.AluOpType.not_equal,
                fill=1.0,
                base=0,
                pattern=[[-1, P]],
                channel_multiplier=1,
            )
            # triangular mask [128,128] bf16: keep (1.0) where q >= kv
            tri = const.tile([P, P], BF16, name="tri")
            nc.gpsimd.memset(tri[:], 1.0)
            nc.gpsimd.affine_select(
                out=tri[:],
                in_=tri[:],
                compare_op=mybir.AluOpType.is_ge,
                fill=0.0,
                base=0,
                pattern=[[1, P]],
                channel_multiplier=-1,
            )
            # ACT exp-table warmup while DMAs run
            warm = const.tile([P, 8], F32, name="warm")
            nc.gpsimd.memset(warm[:], 0.0)
            nc.scalar.activation(out=warm[:], in_=warm[:], func=EXP)

            bq_sb = const.tile([P, 2], F32)
            bk_sb = const.tile([P, 2], F32)
            w0_sb = [const.tile([P, D], BF16, name=f"w0_{kc}") for kc in range(2)]
            # QT/KT as [mi][ni] tiles of [128, 512] for fine-grained deps
            QT = [
                [const.tile([P, 512], BF16, name=f"qt{mi}_{ni}") for ni in range(NQ)]
                for mi in range(2)
            ]
            KT = [
                [const.tile([P, 512], BF16, name=f"kt{mi}_{ni}") for ni in range(NQ)]
                for mi in range(2)
            ]
            # normalized attention (transposed), per q-chunk and head-pair
            attn = [
                [const.tile([P, 512], BF16, name=f"attn{j}_{p}") for p in range(2)]
                for j in range(4)
            ]

            # first k-slice of each weight as its own tile/DMA so the first
            # projection sub-sweep starts as soon as ~130KB has landed
            wq_t0 = wp.tile([P, HD], BF16, name="wq0")
            wk_t0 = wp.tile([P, HD], BF16, name="wk0")
            wq_r = wp.tile([P, NKD - 1, HD], BF16, name="wqr")
            wk_r = wp.tile([P, NKD - 1, HD], BF16, name="wkr")
            wq_view = Wq.rearrange("(k p) c -> p k c", p=P)
            wk_view = Wk.rearrange("(k p) c -> p k c", p=P)
            nc.sync.dma_start(out=wq_t0[:], in_=wq_view[:, 0, :])
            xh = [
                [xtp.tile([P, 1024], BF16, name="xtile") for _ in range(2)]
                for _ in range(NKD)
            ]
            nc.sync.dma_start(out=xh[0][0][:], in_=xT[0:P, 0:1024])
            nc.sync.dma_start(out=wk_t0[:], in_=wk_view[:, 0, :])
            nc.sync.dma_start(out=bq_sb[:], in_=bqt[:, :])
            nc.sync.dma_start(out=bk_sb[:], in_=bkt[:, :])
            nc.sync.dma_start(out=wq_r[:], in_=wq_view[:, 1:NKD, :])
            nc.sync.dma_start(out=wk_r[:], in_=wk_view[:, 1:NKD, :])
            wq_t = [wq_t0[:]] + [wq_r[:, k - 1, :] for k in range(1, NKD)]
            wk_t = [wk_t0[:]] + [wk_r[:, k - 1, :] for k in range(1, NKD)]
            # x half-0 chunks first, then half-1, then W0: the DMA path
            # drains in issue order, which staggers arrivals naturally
            for h in range(2):
                for k in range(NKD):
                    if h == 0 and k == 0:
                        continue
                    nc.sync.dma_start(
                        out=xh[k][h][:],
                        in_=xT[k * P : (k + 1) * P, h * 1024 : (h + 1) * 1024],
                    )
            for kc in range(2):
                nc.sync.dma_start(out=w0_sb[kc][:], in_=W0[kc * P : (kc + 1) * P, :])

            def sweep_items(ni, mi):
                """Projection sub-sweep as a list of emit-thunks (per-k)."""
                half, col = divmod(ni, 2)
                pss = [mmp.tile([P, 512], F32, name="ps") for _ in range(2)]

                def mk(k):
                    def go():
                        for ps, wt in zip(pss, (wq_t, wk_t)):
                            nc.tensor.matmul(
                                ps[:],
                                lhsT=wt[k][:, mi * P : (mi + 1) * P],
                                rhs=xh[k][half][:, col * 512 : (col + 1) * 512],
                                start=(k == 0),
                                stop=(k == NKD - 1),
                            )
                    return go

                def evict():
                    # in the DMA-paced startup ACT is idle: the K eviction of
                    # the first two sweeps runs there so QT/KT land sooner
                    early = ni < 2 and mi == 0
                    for n_, (ps, bias, dstT) in enumerate(
                        zip(pss, (bq_sb, bk_sb), (QT, KT))
                    ):
                        if early and n_ == 1:
                            nc.scalar.add(
                                dstT[mi][ni][:, :], ps[:], bias[:, mi : mi + 1]
                            )
                        else:
                            nc.vector.tensor_scalar_add(
                                dstT[mi][ni][:, :], ps[:], bias[:, mi : mi + 1]
                            )

                return [mk(k) for k in range(NKD)] + [evict]

            vp = {}

            def emit_transposes(pair, i_lo, i_hi):
                # V' tiles [128, 132] via XBAR dma transpose from QT:
                # A data 0:64, A one 64, B data 66:130, B one 130.
                for i in range(i_lo, i_hi):
                    if (pair, i) in vp:
                        continue
                    src = QT[pair][i // 4][:, (i % 4) * P : (i % 4 + 1) * P]
                    # B data sits at col 80: XBAR writes require the out
                    # offset to be 32-byte aligned (16 bf16 cols)
                    vt = vpool.tile([P, 148], BF16, name="vt")
                    nc.sync.dma_start_transpose(out=vt[:, 0:64], in_=src[0:64, :])
                    nc.sync.dma_start_transpose(out=vt[:, 80:144], in_=src[64:128, :])
                    nc.gpsimd.tensor_copy(vt[:, 64:65], ones_col[:])
                    nc.gpsimd.tensor_copy(vt[:, 144:145], ones_col[:])
                    vp[(pair, i)] = vt

            def emit_transposes_pe(pair, i_lo, i_hi):
                # startup variant: PE transposes (PE is idle while DMAs land,
                # and the DMA/HWDGE path is saturated with input loads)
                # shares the "aps" slots: dead before the first atB allocation
                tp = apsum.tile([P, 512], BF16, name="aps")
                for i in range(i_lo, i_hi):
                    c = (i - i_lo) * P
                    nc.tensor.transpose(
                        tp[:, c : c + P],
                        QT[pair][i // 4][:, (i % 4) * P : (i % 4 + 1) * P],
                        identity[:],
                    )
                    vt = vpool.tile([P, 148], BF16, name="vt")
                    nc.vector.tensor_copy(vt[:, 0:64], tp[:, c : c + 64])
                    nc.vector.tensor_copy(vt[:, 80:144], tp[:, c + 64 : c + P])
                    nc.gpsimd.tensor_copy(vt[:, 64:65], ones_col[:])
                    nc.gpsimd.tensor_copy(vt[:, 144:145], ones_col[:])
                    vp[(pair, i)] = vt

            bg = []  # drip queue of (cost, key, thunk)
            carry = [0.0]

            def drip(budget):
                # carry-based pacing: costs are ~213ns PE units; fractional
                # budget accumulates so fill work spreads over the whole
                # attention phase instead of draining the queue early
                carry[0] += budget
                while bg and carry[0] >= bg[0][0]:
                    cost, _key, thunk = bg.pop(0)
                    thunk()
                    carry[0] -= cost

            def ensure(key):
                # need-driven drain: emit every queued item with this key now
                # (deadline reached), regardless of queue position
                rest, hits = [], []
                for item in bg:
                    (hits if item[1] == key else rest).append(item)
                if hits:
                    bg[:] = rest
                    for _cost, _key, thunk in hits:
                        thunk()

            def emit_cblock_m(j, c, act_only=False):
                m = j * 4 + c
                ot = ostp.tile([P, D], BF16, name="ot")
                for n in range(2):
                    ps = mmp.tile([P, 512], F32, name="ps")
                    for kc in range(2):
                        nc.tensor.matmul(
                            ps[:],
                            lhsT=attn[j][kc][:, c * P : (c + 1) * P],
                            rhs=w0_sb[kc][:, n * 512 : (n + 1) * 512],
                            start=(kc == 0),
                            stop=(kc == 1),
                        )
                    osl = slice(n * 512, (n + 1) * 512)
                    # n=0 evicts on ACT to keep DVE clear for the normalize
                    # chain (which gates the next q-chunk's AV psum); in the
                    # endgame (act_only) ACT takes both halves since the DVE
                    # is the pacing engine there
                    if n == 0 or act_only:
                        nc.scalar.copy(ot[:, osl], ps[:])
                    else:
                        nc.vector.tensor_copy(ot[:, osl], ps[:])
                    if act_only:
                        # endgame: per-half DMA so the n=0 half transfers
                        # while the n=1 half is still evicting
                        nc.sync.dma_start(
                            out=out[m * P : (m + 1) * P, osl], in_=ot[:, osl]
                        )
                if not act_only:
                    nc.sync.dma_start(out=out[m * P : (m + 1) * P, :], in_=ot[:])

            def emit_pair(pair):
                steps = [(j, i) for j in range(NQ) for i in range(4 * j + 4)]
                ats = {}

                def emit_S(j, i):
                    off = max(0, i * P - j * 512)  # 0,128,256,384
                    w = 512 - off
                    kc = slice((i % 4) * P, (i % 4 + 1) * P)
                    sAB = spsum.tile([P, 1024], F32, name="sAB")
                    nc.tensor.matmul(
                        sAB[:, off:512],
                        lhsT=KT[pair][i // 4][0:64, kc],
                        rhs=QT[pair][j][0:64, off:512],
                    )
                    nc.tensor.matmul(
                        sAB[:, 512 + off : 1024],
                        lhsT=KT[pair][i // 4][64:128, kc],
                        rhs=QT[pair][j][64:128, off:512],
                    )
                    pAB = ptp.tile([P, 1024], BF16, name="pAB")
                    if w == 512:
                        nc.scalar.activation(out=pAB[:, :], in_=sAB[:, :], func=EXP)
                    else:
                        nc.scalar.activation(
                            out=pAB[:].rearrange("p (t c) -> p t c", t=2)[:, :, off:512],
                            in_=sAB[:].rearrange("p (t c) -> p t c", t=2)[:, :, off:512],
                            func=EXP,
                        )
                    if i >= 4 * j:  # diagonal tile: mask the leading block
                        nc.vector.tensor_mul(
                            pAB[:, off : off + P], pAB[:, off : off + P], tri[:]
                        )
                        nc.vector.tensor_mul(
                            pAB[:, 512 + off : 512 + off + P],
                            pAB[:, 512 + off : 512 + off + P],
                            tri[:],
                        )
                    return (j, i, pAB, off, w)

                def emit_AV(j, i, pAB, off, w):
                    if i == 0:
                        ats[j] = (
                            apsum.tile([P, 512], F32, name="aps"),
                            apsum.tile([P, 512], F32, name="aps"),
                        )
                    atA, atB = ats[j]
                    imax = 4 * j + 3
                    qsl = slice(off, 512)
                    vt = vp[(pair, i)]
                    last = i == imax

                    nc.tensor.matmul(
                        atA[0:65, qsl],
                        lhsT=vt[:, 0:65],
                        rhs=pAB[:, off:512],
                        start=(i == 0),
                        stop=last,
                    )
                    final = pair == 1 and j == NQ - 1
                    recA = None
                    if last and not final:
                        # head A's reciprocal runs while B's AV is on PE
                        recA = dpool.tile([1, 512], F32, name="rec")
                        nc.vector.reciprocal(recA[:], atA[64:65, :])
                    nc.tensor.matmul(
                        atB[0:65, qsl],
                        lhsT=vt[:, 80:145],
                        rhs=pAB[:, 512 + off : 1024],
                        start=(i == 0),
                        stop=last,
                    )
                    if last:
                        if final:
                            # final tile: the whole normalize chain runs at
                            # column-quarter granularity, each cblock emitted
                            # right after the quarter it consumes, so the
                            # first cblock starts ~1us after the last AV
                            rA = dpool.tile([1, 512], F32, name="rec")
                            rB = dpool.tile([1, 512], F32, name="rec")
                            bA = dpool.tile([64, 512], F32, name="rbc")
                            bB = dpool.tile([64, 512], F32, name="rbc")
                            for c in range(4):
                                cs = slice(c * P, (c + 1) * P)
                                nc.vector.reciprocal(rA[:, cs], atA[64:65, cs])
                                nc.vector.reciprocal(rB[:, cs], atB[64:65, cs])
                                nc.gpsimd.partition_broadcast(bA[0:64, cs], rA[0:1, cs])
                                nc.gpsimd.partition_broadcast(bB[0:64, cs], rB[0:1, cs])
                                nc.vector.tensor_mul(
                                    attn[j][pair][0:64, cs], atA[0:64, cs], bA[0:64, cs]
                                )
                                nc.vector.tensor_mul(
                                    attn[j][pair][64:128, cs], atB[0:64, cs], bB[0:64, cs]
                                )
                                emit_cblock_m(j, c, act_only=True)
                            return
                        # normalize attn = att_un / d (d = psum row 64), with
                        # the A and B chains pipelined across DVE and Pool and
                        # the muls in column halves so cblocks unblock early
                        recB = dpool.tile([1, 512], F32, name="rec")
                        nc.vector.reciprocal(recB[:], atB[64:65, :])
                        rbcA = dpool.tile([64, 512], F32, name="rbc")
                        nc.gpsimd.partition_broadcast(rbcA[0:64, :], recA[0:1, :])
                        rbcB = dpool.tile([64, 512], F32, name="rbc")
                        nc.gpsimd.partition_broadcast(rbcB[0:64, :], recB[0:1, :])
                        for h in range(2):
                            cs = slice(h * 256, (h + 1) * 256)
                            nc.vector.tensor_mul(
                                attn[j][pair][0:64, cs], atA[0:64, cs], rbcA[0:64, cs]
                            )
                            nc.vector.tensor_mul(
                                attn[j][pair][64:128, cs], atB[0:64, cs], rbcB[0:64, cs]
                            )
                        if pair == 1:  # output projection becomes available
                            for c in range(4):
                                bg.append(
                                    (4, ("cb", j, c), lambda j=j, c=c: emit_cblock_m(j, c))
                                )

                pend = []
                for j, i in steps:
                    if i == 0:
                        # deadline: this j's QT/KT sweep must be fully emitted
                        ensure(("sw", j, pair))
                    if (pair, i) not in vp:
                        emit_transposes(pair, i, i + 1)
                    pend.append(emit_S(j, i))
                    drip(3 if i <= 2 else 1.5)
                    # depth-2 software pipeline: AV(T) trails S(T) by two
                    # tiles, so the exp on ACT has two tile-periods of slack
                    if len(pend) > 3:
                        emit_AV(*pend.pop(0))
                for p_ in pend:
                    emit_AV(*p_)

            def t_item(pair, i):
                return (0, ("t", pair, i), lambda: emit_transposes(pair, i, i + 1))

            def sweep_bg(ni, mi):
                # per-k items are 2 matmuls of 512 cols (~427ns); evict is DVE
                return [
                    (2 if ix < NKD else 0, ("sw", ni, mi), it)
                    for ix, it in enumerate(sweep_items(ni, mi))
                ]

            # upfront: pair-0 ni=0 projection (DMA-paced) + first V transposes
            for it in sweep_items(0, 0):
                it()
            emit_transposes_pe(0, 0, 4)
            # bg order follows need-by and DMA-arrival order
            bg.extend(sweep_bg(1, 0))
            bg.extend(t_item(0, i) for i in range(4, 8))
            bg.extend(sweep_bg(2, 0))
            bg.extend(t_item(0, i) for i in range(8, 12))
            bg.extend(sweep_bg(3, 0))
            bg.extend(t_item(0, i) for i in range(12, 16))
            bg.extend(sweep_bg(0, 1))
            bg.extend(sweep_bg(1, 1))
            bg.extend(sweep_bg(2, 1))
            bg.extend(sweep_bg(3, 1))
            bg.extend(t_item(1, i) for i in range(0, 16))
            emit_pair(0)
            emit_pair(1)
            while bg:
                drip(8)

    nc.compile()
    return nc


def make_in_maps(pos_encode_toks, Wq, bq, Wk, bk, W0, b0):
    x = np.asarray(pos_encode_toks, dtype=np.float32)
    Wq = np.asarray(Wq, dtype=np.float32)
    bq = np.asarray(bq, dtype=np.float32)
    Wk = np.asarray(Wk, dtype=np.float32)
    bk = np.asarray(bk, dtype=np.float32)
    W0 = np.asarray(W0, dtype=np.float32)
    in_maps = []
    for core in range(8):
        b, g = divmod(core, 4)
        hs = slice(g * HD, (g + 1) * HD)
        scale = np.float32(1.0 / np.sqrt(DK))
        in_maps.append(
            {
                "xT": np.ascontiguousarray(x[b].T).astype(ml_dtypes.bfloat16),
                "Wq": np.ascontiguousarray(Wq[:, hs]).astype(ml_dtypes.bfloat16),
                "Wk": np.ascontiguousarray(Wk[:, hs] * scale).astype(ml_dtypes.bfloat16),
                "bqt": np.ascontiguousarray(bq[hs].reshape(2, P).T),
                "bkt": np.ascontiguousarray((bk[hs] * scale).reshape(2, P).T),
                "W0": np.ascontiguousarray(W0[hs, :]).astype(ml_dtypes.bfloat16),
            }
        )
    return in_maps


def assemble(results, b0):
    out = np.zeros((2, S, D), dtype=np.float32)
    for core in range(8):
        b = core // 4
        out[b] += results[core]["out"].astype(np.float32)
    out += np.asarray(b0, dtype=np.float32)
    return out


def kernel(pos_encode_toks, Wq, bq, Wk, bk, W0, b0):
    from concourse.bass_utils import run_bass_kernel_spmd

    global _CACHED_NC
    if _CACHED_NC is None:
        _CACHED_NC = build_nc()
    in_maps = make_in_maps(pos_encode_toks, Wq, bq, Wk, bk, W0, b0)
    res = run_bass_kernel_spmd(_CACHED_NC, in_maps, core_ids=list(range(8)))
    return assemble(res.results, b0)
